# revision 1
# baseline (speedup 1.0000x reference)
"""AdaptiveInput (adaptive embedding) kernel for 8 TRN2 NeuronCores.

Strategy: data-parallel over tokens (each core takes one batch row of 4096
tokens, embedding tables replicated). The host does only integer index
bookkeeping (compaction of tokens by cluster / vocab sub-range); every
float is touched exclusively on-device:

  per core:  dma_gather rows from DRAM tables -> SBUF (f32)
             PE-transpose each 128-token tile -> PSUM -> bf16 lhsT
             matmul vs bf16 projection weights -> PSUM f32 [128, 1024]
             stage bf16 rows -> dma_scatter_add to out_j[4097, 1024]
             (row 4096 is a trash row: padding slots land there; real
              rows are written exactly once onto the zero-initialized
              outputs, so scatter-ADD == assignment; host widens to f32)

Scatter chunks are group-aligned and rotate across 4 output tensors so
successive scatters have no WAW dependency and their transfers overlap
compute; the host merges the disjoint row sets (unshard-style reassembly).

dma_gather / dma_scatter_add use int16 indices wrapped in 16 partitions,
so vocab ranges larger than 32767 rows are split into sub-range groups.
tail2 rows are only 64B (< the 256B descriptor minimum), so tail2 is
gathered in quad-row chunks (idx = row // 4, 256B) and the unwanted
sub-rows are zeroed after the transpose with a host-provided mask; the
matmul then runs against a 4x-stacked tail_lin2 so the zeroed lanes
contribute nothing.
"""
import sys

if "/opt/trn_rl_repo" not in sys.path:
    sys.path.insert(0, "/opt/trn_rl_repo")

import numpy as np

import concourse.bass as bass
import concourse.tile as tile
from concourse import bacc, mybir
from concourse.bass_utils import run_bass_kernel_spmd

# --- problem constants (hardcoded; kernel.py must be self-contained) ---
N_CORES = 8
N_TOK = 4096                    # tokens per core
D = 1024                        # output feature dim
CUTOFFS = [0, 10000, 60000, 190000, 250000]
HS = [1024, 256, 64, 16]        # embedding width per cluster
SUBRANGE = 32768                # int16 index limit for dma_gather
TRASH_ROW = N_TOK               # padding scatter target
STAGE_TILES = 4                 # output tiles per scatter chunk
N_OUT = 4                       # rotating output tensors
OUT_BF16 = True                # stage/scatter outputs in bf16 (host widens)

F32 = mybir.dt.float32
BF16 = mybir.dt.bfloat16
I16 = mybir.dt.int16


def _plan_groups(tokens_all):
    """Split tokens into gather groups; return group meta + per-core slot data.

    Groups: [head] + [t0 x2 subranges] + [t1 x4 subranges] + [t2-quad].
    Each group g gets cap_g = roundup(max_core_count, 128) slots.
    Slot i of a group <-> (partition i%128, chunk i//128) in SBUF tiles.
    """
    groups = []
    groups.append(dict(cluster=0, lo=0, hi=CUTOFFS[1], quad=False))
    for lo in range(0, CUTOFFS[2] - CUTOFFS[1], SUBRANGE):
        hi = min(lo + SUBRANGE, CUTOFFS[2] - CUTOFFS[1])
        groups.append(dict(cluster=1, lo=CUTOFFS[1] + lo, hi=CUTOFFS[1] + hi, quad=False))
    for lo in range(0, CUTOFFS[3] - CUTOFFS[2], SUBRANGE):
        hi = min(lo + SUBRANGE, CUTOFFS[3] - CUTOFFS[2])
        groups.append(dict(cluster=2, lo=CUTOFFS[2] + lo, hi=CUTOFFS[2] + hi, quad=False))
    groups.append(dict(cluster=3, lo=CUTOFFS[3], hi=CUTOFFS[4], quad=True))

    per_core = []
    for i in range(N_CORES):
        t = tokens_all[i]
        cg = []
        for g in groups:
            sel = np.nonzero((t >= g["lo"]) & (t < g["hi"]))[0]
            loc = t[sel] - g["lo"]
            cg.append((sel.astype(np.int64), loc.astype(np.int64)))
        per_core.append(cg)

    for gi, g in enumerate(groups):
        mx = max(len(per_core[i][gi][0]) for i in range(N_CORES))
        g["mx"] = -(-max(1, mx) // 128) * 128
        g["cap"] = max(128, -(-mx // 128) * 128)
        g["C"] = g["cap"] // 128

    # scatter chunk plan: chunks never span groups; exact idx counts
    chunks = []
    for gi, g in enumerate(groups):
        for t0 in range(0, g["C"], STAGE_TILES):
            ntc = min(STAGE_TILES, g["C"] - t0)
            n_idx = ntc * 128
            chunks.append(dict(gi=gi, t0=t0, ntc=ntc, n_idx=n_idx))
    return groups, per_core, chunks


def _wrap16(vals, cap, pad):
    """vals -> int16 [128, cap//16]: entry i at [i%16, i//16], replicated x8."""
    m = np.full((16, cap // 16), pad, np.int16)
    n = len(vals)
    m[np.arange(n) % 16, np.arange(n) // 16] = vals.astype(np.int16)
    return np.tile(m, (8, 1))


def _build_graph(groups, chunks, C2):
    S_tot = sum(g["cap"] // 16 for g in groups)
    n_tiles_tot = sum(g["C"] for g in groups)
    n_groups = len(groups)

    nc = bacc.Bacc("TRN2", target_bir_lowering=False, debug=False,
                   num_devices=N_CORES, num_swdge_queues=4)

    p_emb = [
        nc.dram_tensor("head_emb", [CUTOFFS[1], 1024], F32, kind="ExternalInput").ap(),
        nc.dram_tensor("tail_emb0", [CUTOFFS[2] - CUTOFFS[1], 256], F32, kind="ExternalInput").ap(),
        nc.dram_tensor("tail_emb1", [CUTOFFS[3] - CUTOFFS[2], 64], F32, kind="ExternalInput").ap(),
        nc.dram_tensor("tail_emb2", [CUTOFFS[4] - CUTOFFS[3], 16], F32, kind="ExternalInput").ap(),
    ]
    p_hwT = nc.dram_tensor("head_wT", [1024, 1024], F32, kind="ExternalInput").ap()
    p_l0 = nc.dram_tensor("tail_lin0", [256, 1024], F32, kind="ExternalInput").ap()
    p_l1 = nc.dram_tensor("tail_lin1", [64, 1024], F32, kind="ExternalInput").ap()
    p_l2 = nc.dram_tensor("tail_lin2", [16, 1024], F32, kind="ExternalInput").ap()
    p_gidx = nc.dram_tensor("gidx", [128, S_tot], I16, kind="ExternalInput").ap()
    p_spos = nc.dram_tensor("spos", [128, S_tot], I16, kind="ExternalInput").ap()
    p_mask = nc.dram_tensor("maskT2", [64, C2 * 128], F32, kind="ExternalInput").ap()
    p_ident = nc.dram_tensor("ident", [128, 128], F32, kind="ExternalInput").ap()
    out_dt = BF16 if OUT_BF16 else F32
    p_out = [
        nc.dram_tensor(f"out{j}", [N_TOK + 1, D], out_dt, kind="ExternalOutput").ap()
        for j in range(N_OUT)
    ]
    p_l2x4 = nc.dram_tensor("l2x4", [64, 1024], F32).ap()  # internal bounce

    with tile.TileContext(nc) as tc:
        from contextlib import ExitStack
        with ExitStack() as ctx:
            cpool = ctx.enter_context(tc.tile_pool(name="const", bufs=1))
            wstg = ctx.enter_context(tc.tile_pool(name="wstg", bufs=2))
            xgpool = ctx.enter_context(tc.tile_pool(name="xg", bufs=1))
            xtpool = ctx.enter_context(tc.tile_pool(name="xt", bufs=4))
            stpool = ctx.enter_context(tc.tile_pool(name="stage", bufs=6))
            pt_pool = ctx.enter_context(tc.tile_pool(name="ptp", bufs=2, space="PSUM"))
            po_pool = ctx.enter_context(tc.tile_pool(name="pop", bufs=3, space="PSUM"))

            ident = cpool.tile([128, 128], F32, tag="ident")


            gidx_sb = cpool.tile([128, S_tot], I16, tag="gidx")
            spos_sb = cpool.tile([128, S_tot], I16, tag="spos")
            mask_sb = cpool.tile([64, C2 * 128], F32, tag="mask")
            nc.sync.dma_start(out=ident[:], in_=p_ident[:])
            nc.sync.dma_start(out=gidx_sb[:], in_=p_gidx[:])
            nc.sync.dma_start(out=spos_sb[:], in_=p_spos[:])

            # ---- gathers (SWDGE, queues 0/1); emission interleaved below ----
            gather_insts = []
            scatter_insts = []
            xg_tiles = [None] * n_groups
            scol_acc = 0
            for gi, g in enumerate(groups):
                g["scol"] = scol_acc
                scol_acc += g["C"] * 8

            def emit_gather(gi):
                g = groups[gi]
                C = g["C"]
                if g["quad"]:
                    h_eff = 64
                    in_ap = p_emb[3].rearrange("(q f) h -> q (f h)", f=4)
                else:
                    h_eff = HS[g["cluster"]]
                    cl = g["cluster"]
                    base = CUTOFFS[cl]
                    in_ap = p_emb[cl][g["lo"] - base:g["hi"] - base]
                xg = xgpool.tile([128, C, h_eff], F32, tag=f"xg{gi}")
                gins = nc.gpsimd.dma_gather(
                    out_ap=xg[:], in_ap=in_ap,
                    idxs_ap=gidx_sb[:, g["scol"]:g["scol"] + C * 8],
                    num_idxs=g["mx"], num_idxs_reg=g["mx"],
                    elem_size=h_eff,
                    queue_num=0,
                )
                gather_insts.append(gins.ins)
                xg_tiles[gi] = (xg, h_eff)

            emit_gather(0)
            emit_gather(1)
            emit_gather(2)

            # ---- weights via scalar-engine HWDGE (own ring) + ACT converts ----
            def load_w(dst_bf_ap, src_ap, shape):
                stg = wstg.tile(shape, F32, tag="wstg")
                nc.scalar.dma_start(out=stg[:], in_=src_ap)
                nc.scalar.copy(out=dst_bf_ap, in_=stg[:])

            hwT_r = p_hwT.rearrange("(k p) d -> p k d", p=128)
            w_head = cpool.tile([128, 8, 1024], BF16, tag="w_head")
            for k in range(8):
                load_w(w_head[:, k, :], hwT_r[:, k, :], [128, 1024])
            w_l0 = cpool.tile([128, 2, 1024], BF16, tag="w_l0")
            for k in range(2):
                load_w(w_l0[:, k, :], p_l0.rearrange("(k p) d -> p k d", p=128)[:, k, :], [128, 1024])
            w_l1 = cpool.tile([64, 1024], BF16, tag="w_l1")
            load_w(w_l1[:], p_l1[:], [64, 1024])
            w_l2 = cpool.tile([64, 1024], BF16, tag="w_l2")
            for j in range(4):
                nc.scalar.dma_start(out=p_l2x4[16 * j:16 * j + 16, :], in_=p_l2[:])
            load_w(w_l2[:], p_l2x4[:], [64, 1024])
            nc.sync.dma_start(out=mask_sb[:], in_=p_mask[:])

            def rhs_for(g, k):
                cl = g["cluster"]
                if cl == 0:
                    return lambda sl: w_head[:, k, sl]
                if cl == 1:
                    return lambda sl: w_l0[:, k, sl]
                if cl == 2:
                    return lambda sl: w_l1[:, sl]
                return lambda sl: w_l2[:, sl]

            copy_alt = 0
            stage = None
            chunk_i = 0
            next_gather = 3
            for gi, g in enumerate(groups):
                if next_gather < n_groups:
                    emit_gather(next_gather)
                    next_gather += 1
                xg, h_eff = xg_tiles[gi]
                K = -(-h_eff // 128)
                for c in range(g["C"]):
                    xts = []
                    for k in range(K):
                        kk = min(128, h_eff - 128 * k)
                        tps = pt_pool.tile([128, 128], F32, tag="tps")
                        nc.tensor.transpose(
                            out=tps[:kk, :],
                            in_=xg[:, c, 128 * k:128 * k + kk],
                            identity=ident[:],
                        )
                        xt = xtpool.tile([128, 128], BF16, tag="xt")
                        if g["quad"]:
                            nc.vector.tensor_tensor(
                                out=xt[:kk, :], in0=tps[:kk, :],
                                in1=mask_sb[:, 128 * c:128 * (c + 1)],
                                op=mybir.AluOpType.mult,
                            )
                        else:
                            nc.vector.tensor_copy(out=xt[:kk, :], in_=tps[:kk, :])
                        xts.append((xt, kk))

                    po = po_pool.tile([128, 1024], F32, tag="po")
                    for k, (xt, kk) in enumerate(xts):
                        wk = rhs_for(g, k)
                        for n in range(2):
                            sl = slice(512 * n, 512 * (n + 1))
                            nc.tensor.matmul(
                                out=po[:, sl], lhsT=xt[:kk, :], rhs=wk(sl),
                                start=(k == 0), stop=(k == K - 1),
                            )

                    ck = chunks[chunk_i]
                    slot = c - ck["t0"]
                    if slot == 0:
                        stage = stpool.tile([128, STAGE_TILES, 1024], out_dt, tag="stage")
                    if copy_alt % 2 == 0:
                        nc.vector.tensor_copy(out=stage[:, slot, :], in_=po[:])
                    else:
                        nc.scalar.copy(out=stage[:, slot, :], in_=po[:])
                    copy_alt += 1

                    if slot == ck["ntc"] - 1:
                        col0 = g["scol"] + ck["t0"] * 8
                        sins = nc.gpsimd.dma_scatter_add(
                            out_ap=p_out[chunk_i % N_OUT][:],
                            in_ap=stage[:, :ck["ntc"], :],
                            idxs_ap=spos_sb[:, col0:col0 + ck["ntc"] * 8],
                            num_idxs=ck["n_idx"], num_idxs_reg=ck["n_idx"],
                            elem_size=D,
                            queue_num=1 + chunk_i % 3,
                        )
                        scatter_insts.append(sins.ins)
                        chunk_i += 1

            pass

    nc.compile()
    return nc


_GRAPH_CACHE = {}


def _prepare(tokens_all):
    groups, per_core, chunks = _plan_groups(tokens_all)
    C2 = groups[-1]["C"]

    key = tuple((g["cap"], g["mx"]) for g in groups)
    if key not in _GRAPH_CACHE:
        _GRAPH_CACHE[key] = _build_graph(groups, chunks, C2)
    nc = _GRAPH_CACHE[key]

    gidx_np, spos_np, mask_np, merge_np, cnt_np = [], [], [], [], []
    for i in range(N_CORES):
        gcols, scols = [], []
        cnts = np.zeros((1, 16), np.int32)
        mask = np.zeros((64, C2 * 128), np.float32)
        # which output tensor owns each position (by scatter chunk plan)
        pos_by_out = [[] for _ in range(N_OUT)]
        for gi, g in enumerate(groups):
            sel, loc = per_core[i][gi]
            if g["quad"]:
                gvals = loc // 4
                sub = loc % 4
                for s_i, ssub in enumerate(sub):
                    p, c = s_i % 128, s_i // 128
                    mask[16 * ssub:16 * (ssub + 1), 128 * c + p] = 1.0
            else:
                gvals = loc
            gcols.append(_wrap16(gvals, g["cap"], 0))
            scols.append(_wrap16(sel, g["cap"], TRASH_ROW))
            cnts[0, gi] = len(sel)
        for ci, ck in enumerate(chunks):
            sel = per_core[i][ck["gi"]][0]
            a = 128 * ck["t0"]
            b = min(len(sel), a + ck["n_idx"])
            if b > a:
                pos_by_out[ci % N_OUT].extend(sel[a:b])
        gidx_np.append(np.concatenate(gcols, axis=1))
        spos_np.append(np.concatenate(scols, axis=1))
        mask_np.append(mask)
        merge_np.append([np.asarray(p, np.int64) for p in pos_by_out])
        cnt_np.append(cnts)
    return nc, groups, gidx_np, spos_np, mask_np, merge_np, cnt_np


def run(inputs, trace=False):
    tokens = np.asarray(inputs["tokens"])
    tokens_all = tokens.reshape(N_CORES, N_TOK).astype(np.int64)
    nc, groups, gidx_np, spos_np, mask_np, merge_np, cnt_np = _prepare(tokens_all)

    head_wT = np.ascontiguousarray(np.asarray(inputs["head_w"]).T)
    shared = {
        "head_emb": np.asarray(inputs["head_emb"], np.float32),
        "tail_emb0": np.asarray(inputs["tail_emb0"], np.float32),
        "tail_emb1": np.asarray(inputs["tail_emb1"], np.float32),
        "tail_emb2": np.asarray(inputs["tail_emb2"], np.float32),
        "head_wT": head_wT.astype(np.float32),
        "tail_lin0": np.asarray(inputs["tail_lin0"], np.float32),
        "tail_lin1": np.asarray(inputs["tail_lin1"], np.float32),
        "tail_lin2": np.asarray(inputs["tail_lin2"], np.float32),
    }
    in_maps = []
    for i in range(N_CORES):
        m = dict(shared)
        m["gidx"] = gidx_np[i]
        m["spos"] = spos_np[i]
        m["maskT2"] = mask_np[i]
        m["ident"] = np.eye(128, dtype=np.float32)
        in_maps.append(m)

    res = None
    for attempt in range(3):
        try:
            res = run_bass_kernel_spmd(nc, in_maps, core_ids=list(range(N_CORES)),
                                       trace=trace)
            break
        except Exception:
            if attempt == 2:
                raise
            import time
            time.sleep(2)
    out = np.empty((N_CORES, N_TOK, D), np.float32)
    for i in range(N_CORES):
        for j in range(N_OUT):
            pos = merge_np[i][j]
            if len(pos):
                out[i][pos] = res.results[i][f"out{j}"][pos].astype(np.float32)
    return out, res


def kernel(**inputs):
    out, _ = run(inputs, trace=False)
    return out



# revision 4
# speedup vs baseline: 1.3904x; 1.3904x over previous
"""AdaptiveInput (adaptive embedding) kernel for 8 TRN2 NeuronCores.

v2 strategy (trace-driven rewrite of the scatter-add baseline):

  - Host deals tokens to cores round-robin PER GROUP (stratified), so every
    core gets an equal share of each vocab sub-range: tight static caps,
    balanced cores.  Host work is integer bookkeeping only; every float is
    touched exclusively on-device.
  - Groups are processed tail-first (tail1 x4 subranges, tail2-quad,
    tail0 x2, head LAST) so the 4MB head weight DMA overlaps tail compute.
  - Gathers: SWDGE dma_gather on queues 0-3, padding slots = -1 (trailing
    negative indices are skipped by the Q7 desc-gen -> no padded traffic).
  - Weights: HWDGE f32 loads on the scalar ring + ACT bf16 converts,
    emitted with lookahead so loads/converts overlap.
  - Compute per 128-token tile: PE transposes batched 4-to-a-PSUM-bank,
    ONE DVE cast-copy per batch (f32 PSUM -> bf16 SBUF lhsT), bf16
    matmuls vs resident weights, stage copy PSUM -> bf16 SBUF
    (alternating DVE/ACT).
  - Output: NO scatter.  Tiles are staged [128, ST, 1024] bf16 and written
    with plain contiguous HWDGE dma_start on the sync ring to a
    partition-major DRAM tensor out[128, T_total, 1024] (slot s of group g
    lives at partition s%128, tile tile0_g + s//128).  8KB descriptors,
    no read-modify-write, no SWDGE desc-gen.  Host unpermutes rows.
"""
import sys

if "/opt/trn_rl_repo" not in sys.path:
    sys.path.insert(0, "/opt/trn_rl_repo")

import numpy as np

import concourse.bass as bass
import concourse.tile as tile
from concourse import bacc, mybir
from concourse.bass_utils import run_bass_kernel_spmd

# --- problem constants (hardcoded; kernel.py must be self-contained) ---
N_CORES = 8
N_TOK = 4096                    # tokens per core (8 x 4096 total)
D = 1024                        # output feature dim
CUTOFFS = [0, 10000, 60000, 190000, 250000]
HS = [1024, 256, 64, 16]        # embedding width per cluster
SUBRANGE = 32768                # int16 index limit for dma_gather
ST = 4                          # output tiles per contiguous write chunk
PAD_IDX = -1                    # trailing negative gather idx => skipped

F32 = mybir.dt.float32
BF16 = mybir.dt.bfloat16
I16 = mybir.dt.int16


def _make_groups():
    """Groups in processing order: tail1 x4, tail2-quad, tail0 x2, head."""
    groups = []
    base = CUTOFFS[2]
    for lo in range(0, CUTOFFS[3] - CUTOFFS[2], SUBRANGE):
        hi = min(lo + SUBRANGE, CUTOFFS[3] - CUTOFFS[2])
        groups.append(dict(cluster=2, lo=base + lo, hi=base + hi, quad=False))
    groups.append(dict(cluster=3, lo=CUTOFFS[3], hi=CUTOFFS[4], quad=True))
    base = CUTOFFS[1]
    for lo in range(0, CUTOFFS[2] - CUTOFFS[1], SUBRANGE):
        hi = min(lo + SUBRANGE, CUTOFFS[2] - CUTOFFS[1])
        groups.append(dict(cluster=1, lo=base + lo, hi=base + hi, quad=False))
    groups.append(dict(cluster=0, lo=0, hi=CUTOFFS[1], quad=False))
    return groups


def _plan(tokens_flat):
    """Stratified deal: group tokens by vocab sub-range, round-robin cores."""
    groups = _make_groups()
    per_core = [[] for _ in range(N_CORES)]
    for g in groups:
        idxs = np.nonzero((tokens_flat >= g["lo"]) & (tokens_flat < g["hi"]))[0]
        mx = 0
        for i in range(N_CORES):
            sel = idxs[i::N_CORES]
            per_core[i].append((sel, (tokens_flat[sel] - g["lo"]).astype(np.int64)))
            mx = max(mx, len(sel))
        g["cap"] = max(128, -(-mx // 128) * 128)
        g["C"] = g["cap"] // 128
    t0 = 0
    for g in groups:
        g["tile0"] = t0
        t0 += g["C"]
    return groups, per_core, t0


def _wrap16(vals, cap, pad):
    """vals -> int16 [128, cap//16]: entry i at [i%16, i//16], replicated x8."""
    m = np.full((16, cap // 16), pad, np.int16)
    n = len(vals)
    m[np.arange(n) % 16, np.arange(n) // 16] = vals.astype(np.int16)
    return np.tile(m, (8, 1))


def _build_graph(groups, T_total):
    n_groups = len(groups)
    C2 = next(g["C"] for g in groups if g["quad"])
    S_tot = sum(g["cap"] // 16 for g in groups) + 4  # +4 cols: l2-stack idxs

    nc = bacc.Bacc("TRN2", target_bir_lowering=False, debug=False,
                   num_devices=N_CORES, num_swdge_queues=4)

    p_emb = [
        nc.dram_tensor("head_emb", [CUTOFFS[1], 1024], F32, kind="ExternalInput").ap(),
        nc.dram_tensor("tail_emb0", [CUTOFFS[2] - CUTOFFS[1], 256], F32, kind="ExternalInput").ap(),
        nc.dram_tensor("tail_emb1", [CUTOFFS[3] - CUTOFFS[2], 64], F32, kind="ExternalInput").ap(),
        nc.dram_tensor("tail_emb2", [CUTOFFS[4] - CUTOFFS[3], 16], F32, kind="ExternalInput").ap(),
    ]
    p_hwT = nc.dram_tensor("head_wT", [1024, 1024], F32, kind="ExternalInput").ap()
    p_l0 = nc.dram_tensor("tail_lin0", [256, 1024], F32, kind="ExternalInput").ap()
    p_l1 = nc.dram_tensor("tail_lin1", [64, 1024], F32, kind="ExternalInput").ap()
    p_l2 = nc.dram_tensor("tail_lin2", [16, 1024], F32, kind="ExternalInput").ap()
    p_gidx = nc.dram_tensor("gidx", [128, S_tot], I16, kind="ExternalInput").ap()
    p_mask = nc.dram_tensor("maskT2", [64, C2 * 128], F32, kind="ExternalInput").ap()
    p_ident = nc.dram_tensor("ident", [128, 128], F32, kind="ExternalInput").ap()
    p_out = nc.dram_tensor("out", [128, T_total, 1024], BF16, kind="ExternalOutput").ap()

    with tile.TileContext(nc) as tc:
        from contextlib import ExitStack
        with ExitStack() as ctx:
            cpool = ctx.enter_context(tc.tile_pool(name="const", bufs=1))
            xgpool = ctx.enter_context(tc.tile_pool(name="xg", bufs=1))
            xtpool = ctx.enter_context(tc.tile_pool(name="xt", bufs=3))
            stpool = ctx.enter_context(tc.tile_pool(name="stage", bufs=4))
            ptb_pool = ctx.enter_context(tc.tile_pool(name="ptb", bufs=2, space="PSUM"))
            po_pool = ctx.enter_context(tc.tile_pool(name="pop", bufs=3, space="PSUM"))

            ident = cpool.tile([128, 128], F32, tag="ident")
            gidx_sb = cpool.tile([128, S_tot], I16, tag="gidx")
            mask_sb = cpool.tile([64, C2 * 128], F32, tag="mask")
            nc.sync.dma_start(out=gidx_sb[:], in_=p_gidx[:])
            nc.sync.dma_start(out=ident[:], in_=p_ident[:])
            nc.sync.dma_start(out=mask_sb[:], in_=p_mask[:])

            # ---- gathers: SWDGE queues 0-3, all emitted upfront ----------
            scol = 0
            for g in groups:
                g["scol"] = scol
                scol += g["C"] * 8

            # l2-stack gather first (tiny): partitions 0-63 <- l2 row p%16
            w_l2f = cpool.tile([128, 1, 1024], F32, tag="w_l2f")
            nc.gpsimd.dma_gather(
                out_ap=w_l2f[:], in_ap=p_l2[:],
                idxs_ap=gidx_sb[:, scol:scol + 4],
                num_idxs=64, num_idxs_reg=64, elem_size=1024, queue_num=3,
            )

            xg_tiles = []
            for gi, g in enumerate(groups):
                if g["quad"]:
                    h_eff = 64
                    in_ap = p_emb[3].rearrange("(q f) h -> q (f h)", f=4)
                else:
                    h_eff = HS[g["cluster"]]
                    cl = g["cluster"]
                    base = CUTOFFS[cl]
                    in_ap = p_emb[cl][g["lo"] - base:g["hi"] - base]
                xg = xgpool.tile([128, g["C"], h_eff], F32, tag=f"xg{gi}")
                nc.gpsimd.dma_gather(
                    out_ap=xg[:], in_ap=in_ap,
                    idxs_ap=gidx_sb[:, g["scol"]:g["scol"] + g["C"] * 8],
                    num_idxs=g["cap"], num_idxs_reg=g["cap"],
                    elem_size=h_eff,
                    queue_num=gi % 3,
                )
                xg_tiles.append((xg, h_eff))

            # ---- weights: HWDGE f32 loads (scalar ring, ordered by need) --
            # small converts early on ACT; big (l0/head) converts on Pool so
            # the ACT instruction stream never stalls behind the 4MB head
            # load (Pool's converts queue after its gather desc-gens, which
            # is exactly when those weights arrive).
            w_l1 = cpool.tile([64, 1024], BF16, tag="w_l1")
            w_l2 = cpool.tile([64, 1024], BF16, tag="w_l2")
            w_l0 = cpool.tile([128, 2, 1024], BF16, tag="w_l0")
            w_head = cpool.tile([128, 8, 1024], BF16, tag="w_head")
            w_l1_f = cpool.tile([64, 1024], F32, tag="w_l1_f")
            w_l0_f = cpool.tile([128, 2, 1024], F32, tag="w_l0_f")
            w_head_f = cpool.tile([128, 8, 1024], F32, tag="w_head_f")
            hwT_r = p_hwT.rearrange("(k p) d -> p k d", p=128)
            l0_r = p_l0.rearrange("(k p) d -> p k d", p=128)

            nc.scalar.dma_start(out=w_l1_f[:], in_=p_l1[:])
            nc.scalar.dma_start(out=w_l0_f[:], in_=l0_r[:])
            nc.scalar.dma_start(out=w_head_f[:], in_=hwT_r[:])
            nc.scalar.copy(out=w_l1[:], in_=w_l1_f[:])
            nc.scalar.copy(out=w_l2[:], in_=w_l2f[:64, 0, :])
            nc.gpsimd.tensor_copy(out=w_l0[:], in_=w_l0_f[:])
            nc.gpsimd.tensor_copy(out=w_head[:], in_=w_head_f[:])

            def rhs_for(g):
                cl = g["cluster"]
                if cl == 0:
                    return lambda k, sl: w_head[:, k, sl]
                if cl == 1:
                    return lambda k, sl: w_l0[:, k, sl]
                if cl == 2:
                    return lambda k, sl: w_l1[:, sl]
                return lambda k, sl: w_l2[:, sl]

            # ---- compute: batched transposes -> cast-copy -> matmuls -----
            # batch = up to 4 transpose units of [128(tok), kk<=128] each,
            # sharing one 1-bank PSUM tile [128, 512].
            # unit = (gi, c, k): tile c of group gi, contraction slice k.
            units = []
            tiles = []  # (gi, c) in processing order
            for gi, g in enumerate(groups):
                K = -(-xg_tiles[gi][1] // 128)
                g["K"] = K
                for c in range(g["C"]):
                    tiles.append((gi, c))
                    for k in range(K):
                        units.append((gi, c, k))

            # group units into batches (never across groups)
            batches = []
            cur = []
            for u in units:
                if cur and (len(cur) == 4 or cur[0][0] != u[0]):
                    batches.append(cur)
                    cur = []
                cur.append(u)
            if cur:
                batches.append(cur)

            # map (gi,c,k) -> (batch idx, col offset)
            upos = {}
            for bi, b in enumerate(batches):
                for ui, u in enumerate(b):
                    upos[u] = (bi, ui * 128)

            # emit transposes/casts with 1-batch lookahead over matmuls
            xt_of_batch = {}

            def emit_batch(bi):
                b = batches[bi]
                gi = b[0][0]
                xg, h_eff = xg_tiles[gi]
                g = groups[gi]
                kk = min(128, h_eff)
                used = len(b) * 128
                ptb = ptb_pool.tile([128, 512], F32, tag="ptb")
                for ui, (gi_, c, k) in enumerate(b):
                    cw = min(128, h_eff - 128 * k)
                    nc.tensor.transpose(
                        out=ptb[:cw, 128 * ui:128 * ui + 128],
                        in_=xg[:, c, 128 * k:128 * k + cw],
                        identity=ident[:],
                    )
                xt = xtpool.tile([128, 512], BF16, tag="xt")
                if g["quad"]:
                    c0 = b[0][1]
                    nc.vector.tensor_tensor(
                        out=xt[:kk, :used], in0=ptb[:kk, :used],
                        in1=mask_sb[:, 128 * c0:128 * c0 + used],
                        op=mybir.AluOpType.mult,
                    )
                else:
                    nc.vector.tensor_copy(out=xt[:kk, :used], in_=ptb[:kk, :used])
                xt_of_batch[bi] = xt

            copy_alt = 0
            emitted = 0

            def ensure_batch(bi):
                nonlocal emitted
                while emitted <= bi + 1 and emitted < len(batches):
                    emit_batch(emitted)
                    emitted += 1

            stage = None
            chunk0 = 0
            for ti, (gi, c) in enumerate(tiles):
                g = groups[gi]
                xg, h_eff = xg_tiles[gi]
                K = g["K"]
                rhs = rhs_for(g)

                po = po_pool.tile([128, 1024], F32, tag="po")
                for k in range(K):
                    bi, col = upos[(gi, c, k)]
                    ensure_batch(bi)
                    xt = xt_of_batch[bi]
                    kk = min(128, h_eff - 128 * k)
                    for n in range(2):
                        sl = slice(512 * n, 512 * (n + 1))
                        nc.tensor.matmul(
                            out=po[:, sl], lhsT=xt[:kk, col:col + 128],
                            rhs=rhs(k, sl),
                            start=(k == 0), stop=(k == K - 1),
                        )

                slot = ti - chunk0
                if slot == 0:
                    stage = stpool.tile([128, ST, 1024], BF16, tag="stage")
                if copy_alt % 2 == 0:
                    nc.vector.tensor_copy(out=stage[:, slot, :], in_=po[:])
                else:
                    nc.scalar.copy(out=stage[:, slot, :], in_=po[:])
                copy_alt += 1

                if slot == ST - 1 or ti == len(tiles) - 1:
                    nw = slot + 1
                    nc.sync.dma_start(
                        out=p_out[:, chunk0:chunk0 + nw, :],
                        in_=stage[:, :nw, :],
                    )
                    chunk0 = ti + 1

    nc.compile()
    return nc


_GRAPH_CACHE = {}


def _prepare(tokens_flat):
    groups, per_core, T_total = _plan(tokens_flat)

    key = tuple(g["cap"] for g in groups)
    if key not in _GRAPH_CACHE:
        _GRAPH_CACHE[key] = _build_graph(groups, T_total)
    nc = _GRAPH_CACHE[key]

    C2 = next(g["C"] for g in groups if g["quad"])
    gidx_np, mask_np = [], []
    l2_idx = _wrap16(np.arange(64) % 16, 64, 0)
    for i in range(N_CORES):
        gcols = []
        mask = np.zeros((64, C2 * 128), np.float32)
        for gi, g in enumerate(groups):
            sel, loc = per_core[i][gi]
            if g["quad"]:
                gvals = loc // 4
                sub = loc % 4
                for s_i, ssub in enumerate(sub):
                    p, c = s_i % 128, s_i // 128
                    mask[16 * ssub:16 * (ssub + 1), 128 * c + p] = 1.0
            else:
                gvals = loc
            gcols.append(_wrap16(gvals, g["cap"], PAD_IDX))
        gcols.append(l2_idx)
        gidx_np.append(np.concatenate(gcols, axis=1))
        mask_np.append(mask)
    return nc, groups, per_core, T_total, gidx_np, mask_np


def run(inputs, trace=False):
    tokens = np.asarray(inputs["tokens"])
    tokens_flat = tokens.reshape(-1).astype(np.int64)
    nc, groups, per_core, T_total, gidx_np, mask_np = _prepare(tokens_flat)

    head_wT = np.ascontiguousarray(np.asarray(inputs["head_w"]).T)
    shared = {
        "head_emb": np.asarray(inputs["head_emb"], np.float32),
        "tail_emb0": np.asarray(inputs["tail_emb0"], np.float32),
        "tail_emb1": np.asarray(inputs["tail_emb1"], np.float32),
        "tail_emb2": np.asarray(inputs["tail_emb2"], np.float32),
        "head_wT": head_wT.astype(np.float32),
        "tail_lin0": np.asarray(inputs["tail_lin0"], np.float32),
        "tail_lin1": np.asarray(inputs["tail_lin1"], np.float32),
        "tail_lin2": np.asarray(inputs["tail_lin2"], np.float32),
        "ident": np.eye(128, dtype=np.float32),
    }
    in_maps = []
    for i in range(N_CORES):
        m = dict(shared)
        m["gidx"] = gidx_np[i]
        m["maskT2"] = mask_np[i]
        in_maps.append(m)

    res = None
    for attempt in range(3):
        try:
            res = run_bass_kernel_spmd(nc, in_maps, core_ids=list(range(N_CORES)),
                                       trace=trace)
            break
        except Exception:
            if attempt == 2:
                raise
            import time
            time.sleep(2)

    out_flat = np.empty((N_CORES * N_TOK, D), np.float32)
    for i in range(N_CORES):
        r = res.results[i]["out"]  # [128, T_total, 1024] bf16
        for gi, g in enumerate(groups):
            sel, _ = per_core[i][gi]
            n = len(sel)
            if n:
                s = np.arange(n)
                out_flat[sel] = r[s % 128, g["tile0"] + s // 128].astype(np.float32)
    return out_flat.reshape(tokens.shape[0], tokens.shape[1], D), res


def kernel(**inputs):
    out, _ = run(inputs, trace=False)
    return out


# revision 6
# speedup vs baseline: 1.4048x; 1.0103x over previous
"""AdaptiveInput (adaptive embedding) kernel for 8 TRN2 NeuronCores.

v3 strategy (trace-driven):
  - Host deals tokens to cores round-robin PER GROUP (stratified): tight
    static caps, balanced cores.  Host does integer bookkeeping only.
  - Groups processed tail-first (tail1 x4, tail2-quad, tail0 x2, head
    LAST) so the 4MB head weight DMA overlaps tail compute.
  - Gathers: SWDGE dma_gather queues 0-2, pads = -1 (skipped transfers).
  - Weights: HWDGE f32 loads on the scalar ring; small converts on ACT,
    big (l0/head) converts on Pool after its desc-gens.
  - h=64 groups (tail1/tail2) transpose TWO 128-token tiles per PE
    transpose: lhsT pair [128, 128] -> tile A rows 0:64, tile B rows
    64:128; weights stacked x2 [128, 1024] so each tile's matmul uses
    its own 64-row quadrant (PE quad-tile, base partitions match).
  - Transposes batched 4-per-PSUM-bank, ONE DVE cast-copy per batch.
  - Matmuls: single N=1024 instruction per contraction slice.
  - Output: contiguous HWDGE writes (sync ring) of [128, ST, 1024] bf16
    stages into partition-major out[128, T_total, 1024]; host unpermutes.
"""
import sys

if "/opt/trn_rl_repo" not in sys.path:
    sys.path.insert(0, "/opt/trn_rl_repo")

import numpy as np

import concourse.bass as bass
import concourse.tile as tile
from concourse import bacc, mybir
from concourse.bass_utils import run_bass_kernel_spmd

# --- problem constants (hardcoded; kernel.py must be self-contained) ---
N_CORES = 8
N_TOK = 4096
D = 1024
CUTOFFS = [0, 10000, 60000, 190000, 250000]
HS = [1024, 256, 64, 16]
SUBRANGE = 32768
ST = 4                          # output tiles per contiguous write chunk
PAD_IDX = -1
MM_N = 512                      # matmul rhs free size (512 = one PSUM bank)

F32 = mybir.dt.float32
BF16 = mybir.dt.bfloat16
I16 = mybir.dt.int16


def _make_groups():
    groups = []
    base = CUTOFFS[2]
    for lo in range(0, CUTOFFS[3] - CUTOFFS[2], SUBRANGE):
        hi = min(lo + SUBRANGE, CUTOFFS[3] - CUTOFFS[2])
        groups.append(dict(cluster=2, lo=base + lo, hi=base + hi, quad=False))
    groups.append(dict(cluster=3, lo=CUTOFFS[3], hi=CUTOFFS[4], quad=True))
    base = CUTOFFS[1]
    for lo in range(0, CUTOFFS[2] - CUTOFFS[1], SUBRANGE):
        hi = min(lo + SUBRANGE, CUTOFFS[2] - CUTOFFS[1])
        groups.append(dict(cluster=1, lo=base + lo, hi=base + hi, quad=False))
    groups.append(dict(cluster=0, lo=0, hi=CUTOFFS[1], quad=False))
    return groups


def _plan(tokens_flat):
    groups = _make_groups()
    per_core = [[] for _ in range(N_CORES)]
    for g in groups:
        idxs = np.nonzero((tokens_flat >= g["lo"]) & (tokens_flat < g["hi"]))[0]
        mx = 0
        for i in range(N_CORES):
            sel = idxs[i::N_CORES]
            per_core[i].append((sel, (tokens_flat[sel] - g["lo"]).astype(np.int64)))
            mx = max(mx, len(sel))
        g["cap"] = max(128, -(-mx // 128) * 128)
        g["C"] = g["cap"] // 128
    t0 = 0
    for g in groups:
        g["tile0"] = t0
        t0 += g["C"]
    return groups, per_core, t0


def _wrap16(vals, cap, pad):
    m = np.full((16, cap // 16), pad, np.int16)
    n = len(vals)
    m[np.arange(n) % 16, np.arange(n) // 16] = vals.astype(np.int16)
    return np.tile(m, (8, 1))


def _build_graph(groups, T_total):
    C2 = next(g["C"] for g in groups if g["quad"])
    U2 = -(-C2 // 2)  # paired-transpose units for the quad group
    S_tot = sum(g["cap"] // 16 for g in groups) + 8  # +8 cols: l2x2 idxs

    nc = bacc.Bacc("TRN2", target_bir_lowering=False, debug=False,
                   num_devices=N_CORES, num_swdge_queues=4)

    p_emb = [
        nc.dram_tensor("head_emb", [CUTOFFS[1], 1024], F32, kind="ExternalInput").ap(),
        nc.dram_tensor("tail_emb0", [CUTOFFS[2] - CUTOFFS[1], 256], F32, kind="ExternalInput").ap(),
        nc.dram_tensor("tail_emb1", [CUTOFFS[3] - CUTOFFS[2], 64], F32, kind="ExternalInput").ap(),
        nc.dram_tensor("tail_emb2", [CUTOFFS[4] - CUTOFFS[3], 16], F32, kind="ExternalInput").ap(),
    ]
    p_hwT = nc.dram_tensor("head_wT", [1024, 1024], F32, kind="ExternalInput").ap()
    p_l0 = nc.dram_tensor("tail_lin0", [256, 1024], F32, kind="ExternalInput").ap()
    p_l1 = nc.dram_tensor("tail_lin1", [64, 1024], F32, kind="ExternalInput").ap()
    p_l2 = nc.dram_tensor("tail_lin2", [16, 1024], F32, kind="ExternalInput").ap()
    p_gidx = nc.dram_tensor("gidx", [128, S_tot], I16, kind="ExternalInput").ap()
    p_mask = nc.dram_tensor("maskT2", [128, U2 * 128], F32, kind="ExternalInput").ap()
    p_ident = nc.dram_tensor("ident", [128, 128], F32, kind="ExternalInput").ap()
    p_out = nc.dram_tensor("out", [128, T_total, 1024], BF16, kind="ExternalOutput").ap()

    with tile.TileContext(nc) as tc:
        from contextlib import ExitStack
        with ExitStack() as ctx:
            cpool = ctx.enter_context(tc.tile_pool(name="const", bufs=1))
            xgpool = ctx.enter_context(tc.tile_pool(name="xg", bufs=1))
            xtpool = ctx.enter_context(tc.tile_pool(name="xt", bufs=3))
            stpool = ctx.enter_context(tc.tile_pool(name="stage", bufs=4))
            ptb_pool = ctx.enter_context(tc.tile_pool(name="ptb", bufs=2, space="PSUM"))
            po_pool = ctx.enter_context(tc.tile_pool(name="pop", bufs=3, space="PSUM"))

            ident = cpool.tile([128, 128], F32, tag="ident")
            gidx_sb = cpool.tile([128, S_tot], I16, tag="gidx")
            mask_sb = cpool.tile([128, U2 * 128], F32, tag="mask")
            nc.sync.dma_start(out=gidx_sb[:], in_=p_gidx[:])
            nc.sync.dma_start(out=ident[:], in_=p_ident[:])
            nc.sync.dma_start(out=mask_sb[:], in_=p_mask[:])

            scol = 0
            for g in groups:
                g["scol"] = scol
                scol += g["C"] * 8

            # l2 stacked-x2 gather (tiny, first): partition p <- l2 row p%16
            w_l2f = cpool.tile([128, 1, 1024], F32, tag="w_l2f")
            nc.gpsimd.dma_gather(
                out_ap=w_l2f[:], in_ap=p_l2[:],
                idxs_ap=gidx_sb[:, scol:scol + 8],
                num_idxs=128, num_idxs_reg=128, elem_size=1024, queue_num=3,
            )

            xg_tiles = []
            for gi, g in enumerate(groups):
                if g["quad"]:
                    h_eff = 64
                    in_ap = p_emb[3].rearrange("(q f) h -> q (f h)", f=4)
                else:
                    h_eff = HS[g["cluster"]]
                    cl = g["cluster"]
                    base = CUTOFFS[cl]
                    in_ap = p_emb[cl][g["lo"] - base:g["hi"] - base]
                xg = xgpool.tile([128, g["C"], h_eff], F32, tag=f"xg{gi}")
                nc.gpsimd.dma_gather(
                    out_ap=xg[:], in_ap=in_ap,
                    idxs_ap=gidx_sb[:, g["scol"]:g["scol"] + g["C"] * 8],
                    num_idxs=g["cap"], num_idxs_reg=g["cap"],
                    elem_size=h_eff,
                    queue_num=gi % 3,
                )
                xg_tiles.append((xg, h_eff))

            # ---- weights -------------------------------------------------
            w_l1 = cpool.tile([128, 1024], BF16, tag="w_l1")    # stacked x2
            w_l2 = cpool.tile([128, 1024], BF16, tag="w_l2")    # stacked x2
            w_l0 = cpool.tile([128, 2, 1024], BF16, tag="w_l0")
            w_head = cpool.tile([128, 8, 1024], BF16, tag="w_head")
            w_l1_f = cpool.tile([128, 1024], F32, tag="w_l1_f")
            w_l0_f = cpool.tile([128, 2, 1024], F32, tag="w_l0_f")
            w_head_f = cpool.tile([128, 8, 1024], F32, tag="w_head_f")
            hwT_r = p_hwT.rearrange("(k p) d -> p k d", p=128)
            l0_r = p_l0.rearrange("(k p) d -> p k d", p=128)

            nc.scalar.dma_start(out=w_l1_f[0:64, :], in_=p_l1[:])
            nc.scalar.dma_start(out=w_l1_f[64:128, :], in_=p_l1[:])
            nc.scalar.dma_start(out=w_l0_f[:], in_=l0_r[:])
            nc.scalar.dma_start(out=w_head_f[:], in_=hwT_r[:])
            nc.scalar.copy(out=w_l1[:], in_=w_l1_f[:])
            nc.scalar.copy(out=w_l2[:], in_=w_l2f[:, 0, :])
            nc.gpsimd.tensor_copy(out=w_l0[:], in_=w_l0_f[:])
            nc.gpsimd.tensor_copy(out=w_head[:], in_=w_head_f[:])

            def rhs_for(g, sub):
                cl = g["cluster"]
                if cl == 0:
                    return lambda k, sl: w_head[:, k, sl]
                if cl == 1:
                    return lambda k, sl: w_l0[:, k, sl]
                if cl == 2:
                    return lambda k, sl: w_l1[64 * sub:64 * sub + 64, sl]
                return lambda k, sl: w_l2[64 * sub:64 * sub + 64, sl]

            # ---- transpose units ----------------------------------------
            # h=64 groups: unit u = tile pair (2u, 2u+1) -> one [128,128]
            #   transpose (tile A rows 0:64, tile B rows 64:128).
            # h>=128 groups: unit = (tile c, k-slice) -> [128,128].
            units = []   # (gi, u_or_c, k, ncols)
            tiles = []   # (gi, c)
            for gi, g in enumerate(groups):
                h_eff = xg_tiles[gi][1]
                K = -(-h_eff // 128)
                g["K"] = K
                g["paired"] = h_eff == 64
                for c in range(g["C"]):
                    tiles.append((gi, c))
                if g["paired"]:
                    for u in range(-(-g["C"] // 2)):
                        nt = min(2, g["C"] - 2 * u)
                        units.append((gi, u, 0, nt * 64))
                else:
                    for c in range(g["C"]):
                        for k in range(K):
                            units.append((gi, c, k, 128))

            batches = []
            cur = []
            for u in units:
                if cur and (len(cur) == 4 or cur[0][0] != u[0]):
                    batches.append(cur)
                    cur = []
                cur.append(u)
            if cur:
                batches.append(cur)

            upos = {}
            for bi, b in enumerate(batches):
                for ui, u in enumerate(b):
                    upos[(u[0], u[1], u[2])] = (bi, ui * 128)

            xt_of_batch = {}

            def emit_batch(bi):
                b = batches[bi]
                gi = b[0][0]
                xg, h_eff = xg_tiles[gi]
                g = groups[gi]
                kk = 128 if g["paired"] else min(128, h_eff)
                used = len(b) * 128
                ptb = ptb_pool.tile([128, 512], F32, tag="ptb")
                for ui, (gi_, uc, k, ncols) in enumerate(b):
                    if g["paired"]:
                        nt = ncols // 64
                        in_ = xg[:, 2 * uc:2 * uc + nt, :]
                    else:
                        cw = min(128, h_eff - 128 * k)
                        in_ = xg[:, uc, 128 * k:128 * k + cw]
                        ncols = cw
                    nc.tensor.transpose(
                        out=ptb[:ncols, 128 * ui:128 * ui + 128],
                        in_=in_,
                        identity=ident[:],
                    )
                xt = xtpool.tile([128, 512], BF16, tag="xt")
                if g["quad"]:
                    u0 = b[0][1]
                    nc.vector.tensor_tensor(
                        out=xt[:kk, :used], in0=ptb[:kk, :used],
                        in1=mask_sb[:, 128 * u0:128 * u0 + used],
                        op=mybir.AluOpType.mult,
                    )
                else:
                    nc.vector.tensor_copy(out=xt[:kk, :used], in_=ptb[:kk, :used])
                xt_of_batch[bi] = xt

            emitted = 0

            def ensure_batch(bi):
                nonlocal emitted
                while emitted <= bi + 1 and emitted < len(batches):
                    emit_batch(emitted)
                    emitted += 1

            copy_alt = 0
            stage = None
            chunk0 = 0
            for ti, (gi, c) in enumerate(tiles):
                g = groups[gi]
                xg, h_eff = xg_tiles[gi]
                K = g["K"]

                po = po_pool.tile([128, 1024], F32, tag="po")
                if g["paired"]:
                    u, sub = c // 2, c % 2
                    bi, col = upos[(gi, u, 0)]
                    ensure_batch(bi)
                    xt = xt_of_batch[bi]
                    rhs = rhs_for(g, sub)
                    lo = 64 * sub
                    for n in range(1024 // MM_N):
                        sl = slice(MM_N * n, MM_N * (n + 1))
                        nc.tensor.matmul(
                            out=po[:, sl], lhsT=xt[lo:lo + 64, col:col + 128],
                            rhs=rhs(0, sl), start=True, stop=True,
                        )
                else:
                    rhs = rhs_for(g, 0)
                    for k in range(K):
                        bi, col = upos[(gi, c, k)]
                        ensure_batch(bi)
                        xt = xt_of_batch[bi]
                        for n in range(1024 // MM_N):
                            sl = slice(MM_N * n, MM_N * (n + 1))
                            nc.tensor.matmul(
                                out=po[:, sl], lhsT=xt[:128, col:col + 128],
                                rhs=rhs(k, sl),
                                start=(k == 0), stop=(k == K - 1),
                            )

                slot = ti - chunk0
                if slot == 0:
                    stage = stpool.tile([128, ST, 1024], BF16, tag="stage")
                if copy_alt % 2 == 0:
                    nc.vector.tensor_copy(out=stage[:, slot, :], in_=po[:])
                else:
                    nc.scalar.copy(out=stage[:, slot, :], in_=po[:])
                copy_alt += 1

                if slot == ST - 1 or ti == len(tiles) - 1:
                    nw = slot + 1
                    nc.sync.dma_start(
                        out=p_out[:, chunk0:chunk0 + nw, :],
                        in_=stage[:, :nw, :],
                    )
                    chunk0 = ti + 1

    nc.compile()
    return nc


_GRAPH_CACHE = {}


def _prepare(tokens_flat):
    groups, per_core, T_total = _plan(tokens_flat)

    key = tuple(g["cap"] for g in groups)
    if key not in _GRAPH_CACHE:
        _GRAPH_CACHE[key] = _build_graph(groups, T_total)
    nc = _GRAPH_CACHE[key]

    C2 = next(g["C"] for g in groups if g["quad"])
    U2 = -(-C2 // 2)
    gidx_np, mask_np = [], []
    l2_idx = _wrap16(np.arange(128) % 16, 128, 0)
    for i in range(N_CORES):
        gcols = []
        mask = np.zeros((128, U2 * 128), np.float32)
        for gi, g in enumerate(groups):
            sel, loc = per_core[i][gi]
            if g["quad"]:
                gvals = loc // 4
                sub = loc % 4
                for s_i, ssub in enumerate(sub):
                    p, c = s_i % 128, s_i // 128
                    row = 64 * (c % 2) + 16 * ssub
                    mask[row:row + 16, 128 * (c // 2) + p] = 1.0
            else:
                gvals = loc
            gcols.append(_wrap16(gvals, g["cap"], PAD_IDX))
        gcols.append(l2_idx)
        gidx_np.append(np.concatenate(gcols, axis=1))
        mask_np.append(mask)
    return nc, groups, per_core, T_total, gidx_np, mask_np


def run(inputs, trace=False):
    tokens = np.asarray(inputs["tokens"])
    tokens_flat = tokens.reshape(-1).astype(np.int64)
    nc, groups, per_core, T_total, gidx_np, mask_np = _prepare(tokens_flat)

    head_wT = np.ascontiguousarray(np.asarray(inputs["head_w"]).T)
    shared = {
        "head_emb": np.asarray(inputs["head_emb"], np.float32),
        "tail_emb0": np.asarray(inputs["tail_emb0"], np.float32),
        "tail_emb1": np.asarray(inputs["tail_emb1"], np.float32),
        "tail_emb2": np.asarray(inputs["tail_emb2"], np.float32),
        "head_wT": head_wT.astype(np.float32),
        "tail_lin0": np.asarray(inputs["tail_lin0"], np.float32),
        "tail_lin1": np.asarray(inputs["tail_lin1"], np.float32),
        "tail_lin2": np.asarray(inputs["tail_lin2"], np.float32),
        "ident": np.eye(128, dtype=np.float32),
    }
    in_maps = []
    for i in range(N_CORES):
        m = dict(shared)
        m["gidx"] = gidx_np[i]
        m["maskT2"] = mask_np[i]
        in_maps.append(m)

    res = None
    for attempt in range(3):
        try:
            res = run_bass_kernel_spmd(nc, in_maps, core_ids=list(range(N_CORES)),
                                       trace=trace)
            break
        except Exception:
            if attempt == 2:
                raise
            import time
            time.sleep(2)

    out_flat = np.empty((N_CORES * N_TOK, D), np.float32)
    for i in range(N_CORES):
        r = res.results[i]["out"]  # [128, T_total, 1024] bf16
        for gi, g in enumerate(groups):
            sel, _ = per_core[i][gi]
            n = len(sel)
            if n:
                s = np.arange(n)
                out_flat[sel] = r[s % 128, g["tile0"] + s // 128].astype(np.float32)
    return out_flat.reshape(tokens.shape[0], tokens.shape[1], D), res


def kernel(**inputs):
    out, _ = run(inputs, trace=False)
    return out
